# revision 12
# baseline (speedup 1.0000x reference)
"""Trainium2 Bass kernel for nn_DAttentionBaseline (deformable attention).

Self-contained: hardcodes all shapes. kernel(**inputs) -> np.ndarray.

Sharding: data-parallel over batch B=4 x spatial-half (2 cores per batch).
Each core computes: full q (offset path, duplicated within the pair),
offsets/sampling/k/vT (duplicated), attention + lepe + out-proj for its
own 28-row half of the 56x56 image. No collectives; host concatenates.
"""
import sys
sys.path.insert(0, '/opt/trn_rl_repo')
from contextlib import ExitStack

import numpy as np

import concourse.bass as bass
import concourse.tile as tile
from concourse import bacc, mybir

F32 = mybir.dt.float32
F32R = mybir.dt.float32r
BF16 = mybir.dt.bfloat16
FP16 = mybir.dt.float16
I32 = mybir.dt.int32
AF = mybir.ActivationFunctionType
ALU = mybir.AluOpType

B, C, H, W = 4, 256, 56, 56
G = 4
NGC = 64
HEADS = 8
HC = 32
STR_DIM = 64
SGC = 16
SCALE = HC ** -0.5
EPS = 1e-5
HW = H * W            # 3136
HK = WK = 28
N = HK * WK           # 784 sample points per group
MH = HW // 2          # 1568 own-half positions
MC = 392              # m-chunk (7 image rows)
NMC = MH // MC        # 4
TROWS = G * HW + 8    # gather table rows (padded)
NSZ = [128] * 6 + [16]  # n-chunk sizes (784 = 6*128 + 16)
TAPS = [(1, 1)] + [(dy, dx) for dy in range(3) for dx in range(3) if (dy, dx) != (1, 1)]


def build_program():
    nc = bacc.Bacc("TRN2", target_bir_lowering=False, debug=False, num_devices=8)

    XF = nc.dram_tensor("XF", [C, HW], F32R, kind="ExternalInput").ap()
    XH = nc.dram_tensor("XH", [C, 30 * W], F32R, kind="ExternalInput").ap()
    XT2 = nc.dram_tensor("XT2", [TROWS, 128], F32, kind="ExternalInput").ap()
    SZ = nc.dram_tensor("SZ", [STR_DIM, 58], F32R, kind="ExternalInput").ap()
    WQT = nc.dram_tensor("WQT", [C, C], F32R, kind="ExternalInput").ap()
    WKTB = nc.dram_tensor("WKTB", [G, 65, C], F32R, kind="ExternalInput").ap()
    WVTB = nc.dram_tensor("WVTB", [G, 65, C], F32R, kind="ExternalInput").ap()
    WOT = nc.dram_tensor("WOT", [C, C], F32R, kind="ExternalInput").ap()
    BQ = nc.dram_tensor("BQ", [C, 1], F32, kind="ExternalInput").ap()
    BO = nc.dram_tensor("BO", [C, 1], F32, kind="ExternalInput").ap()
    RPEB = nc.dram_tensor("RPEB", [C, 1], F32, kind="ExternalInput").ap()
    CODIAG = nc.dram_tensor("CODIAG", [128, 18 * 128], F32R, kind="ExternalInput").ap()
    RPEDIAG = nc.dram_tensor("RPEDIAG", [128, 18 * 128], F32R, kind="ExternalInput").ap()
    COLN = nc.dram_tensor("COLN", [128, 2], F32, kind="ExternalInput").ap()
    COPW = nc.dram_tensor("COPW", [128, 8], F32R, kind="ExternalInput").ap()
    ABDIAG = nc.dram_tensor("ABDIAG", [64, 3 * 64], F32R, kind="ExternalInput").ap()
    ABLN = nc.dram_tensor("ABLN", [64, 2], F32, kind="ExternalInput").ap()
    ABPW = nc.dram_tensor("ABPW", [64, 8], F32R, kind="ExternalInput").ap()
    IND2 = nc.dram_tensor("IND2", [128, 2], F32R, kind="ExternalInput").ap()
    E2 = nc.dram_tensor("E2", [2, 128], F32R, kind="ExternalInput").ap()
    IND416 = nc.dram_tensor("IND416", [64, 4], F32R, kind="ExternalInput").ap()
    E416 = nc.dram_tensor("E416", [4, 64], F32R, kind="ExternalInput").ap()
    ONESUM = nc.dram_tensor("ONESUM", [128, 32], FP16, kind="ExternalInput").ap()
    REF4 = nc.dram_tensor("REF4", [4, N], F32, kind="ExternalInput").ap()
    GBASE = nc.dram_tensor("GBASE", [112, 28], F32, kind="ExternalInput").ap()
    IDN = nc.dram_tensor("IDN", [128, 128], F32, kind="ExternalInput").ap()
    EPSC = nc.dram_tensor("EPSC", [128, 1], F32, kind="ExternalInput").ap()
    BQROW = nc.dram_tensor("BQROW", [1, C], F32R, kind="ExternalInput").ap()
    ZROW = nc.dram_tensor("ZROW", [128, 64], F32R, kind="ExternalInput").ap()
    ONESR = nc.dram_tensor("ONESR", [1, G * N], F32R, kind="ExternalInput").ap()
    XB = nc.dram_tensor("XB", [1, 30 * 56], F32R, kind="ExternalInput").ap()

    Y = nc.dram_tensor("Y", [C, MH], F32, kind="ExternalOutput").ap()

    with tile.TileContext(nc) as tc, ExitStack() as ctx:
        sb = ctx.enter_context(tc.tile_pool(name="sb", bufs=1))
        ctx.enter_context(nc.allow_low_precision(reason="f32r matmul pipeline"))
        sb2 = ctx.enter_context(tc.tile_pool(name="sb2", bufs=2))
        ps = ctx.enter_context(tc.tile_pool(name="ps", bufs=1, space="PSUM"))

        def pst(name, tag, w=MC, p=128):
            return ps.tile([p, w], F32, name=name, tag=tag, padded_shape=[128, 448])

        def load(name, shape, dt_, src, tag=""):
            t = sb.tile(shape, dt_, name=name, tag=tag, bufs=(2 if tag else None)) if tag else sb.tile(shape, dt_, name=name)
            nc.sync.dma_start(t[:], src)
            return t

        xf = [load(f"xf{i}", [128, HW], F32R, XF[128 * i:128 * (i + 1), :], tag="bigx") for i in range(2)]
        xh = [load(f"xh{i}", [128, 30 * W], F32R, XH[128 * i:128 * (i + 1), :]) for i in range(2)]
        sz = load("sz", [64, 58], F32R, SZ[:])
        wqt = [load(f"wqt{i}", [128, C], F32R, WQT[128 * i:128 * (i + 1), :]) for i in range(2)]
        wktb = [load(f"wktb{g}", [65, C], F32R, WKTB[g]) for g in range(G)]
        wvtb = [load(f"wvtb{g}", [65, C], F32R, WVTB[g]) for g in range(G)]
        wot = [load(f"wot{i}", [128, C], F32R, WOT[128 * i:128 * (i + 1), :]) for i in range(2)]
        bq_ = [load(f"bq{i}", [128, 1], F32, BQ[128 * i:128 * (i + 1)]) for i in range(2)]
        bo_ = [load(f"bo{i}", [128, 1], F32, BO[128 * i:128 * (i + 1)]) for i in range(2)]
        rpeb_ = [load(f"rpeb{i}", [128, 1], F32, RPEB[128 * i:128 * (i + 1)]) for i in range(2)]
        codiag = load("codiag", [128, 18 * 128], F32R, CODIAG[:])
        rpediag = load("rpediag", [128, 18 * 128], F32R, RPEDIAG[:])
        coln = load("coln", [128, 2], F32, COLN[:])
        copw = load("copw", [128, 8], F32R, COPW[:])
        abdiag = load("abdiag", [64, 3 * 64], F32R, ABDIAG[:])
        abln = load("abln", [64, 2], F32, ABLN[:])
        abpw = load("abpw", [64, 8], F32R, ABPW[:])
        ind2 = load("ind2", [128, 2], F32R, IND2[:])
        e2 = load("e2", [2, 128], F32R, E2[:])
        ind416 = load("ind416", [64, 4], F32R, IND416[:])
        e416 = load("e416", [4, 64], F32R, E416[:])
        onesum = load("onesum", [128, 32], FP16, ONESUM[:])
        ref4 = load("ref4", [4, N], F32, REF4[:])
        gbase = load("gbase", [112, 28], F32, GBASE[:])
        idn = load("idn", [128, 128], F32, IDN[:])
        epsc = load("epsc", [128, 1], F32, EPSC[:])
        bqrow = load("bqrow", [1, C], F32R, BQROW[:])
        zrow = load("zrow", [128, 64], F32R, ZROW[:])
        xb = load("xb", [1, 30 * 56], F32R, XB[:])

        # ------------- q projections (padded layouts: 58-wide cols, pad rows) ----
        # qf_pad: [58 rows x 58 cols] = (y+1, x+1), zeros on border
        # qh_pad: [30 rows x 58 cols] = (halo rows, x+1), zeros on col border
        qf = [sb.tile([128, 58 * 58], F32R, name=f"qf{i}", tag="bigq", bufs=2,
                      padded_shape=[128, 14 * 256]) for i in range(2)]
        qh = [sb.tile([128, 30 * 58], F32R, name=f"qh{i}") for i in range(2)]
        for i in range(2):
            qfv0 = qf[i][:, :].rearrange("p (a b) -> p a b", b=58)
            qhv0 = qh[i][:, :].rearrange("p (a b) -> p a b", b=58)
            nc.sync.dma_start(qfv0[:, 0, :], ZROW[:, :58])
            nc.sync.dma_start(qfv0[:, 57, :], ZROW[:, :58])
            nc.sync.dma_start(qfv0[:, 1:57, 0], ZROW[:, :56])
            nc.sync.dma_start(qfv0[:, 1:57, 57], ZROW[:, :56])
            nc.sync.dma_start(qhv0[:, :, 0], ZROW[:, :30])
            nc.sync.dma_start(qhv0[:, :, 57], ZROW[:, :30])
        for oc in range(2):
            qfv = qf[oc][:, :].rearrange("p (a b) -> p a b", b=58)
            qhv = qh[oc][:, :].rearrange("p (a b) -> p a b", b=58)
            for j in range(7):      # full q: 7 chunks of 8 rows (448 cols)
                p = pst(f"qp_{oc}_{j}", f"psQK{j % 2}", 448)
                for kc in range(2):
                    nc.tensor.matmul(
                        p[:], wqt[kc][:, 128 * oc:128 * (oc + 1)],
                        xf[kc][:, 448 * j:448 * (j + 1)],
                        start=(kc == 0), stop=(kc == 1))
                nc.scalar.activation(qfv[:, 1 + 8 * j:1 + 8 * (j + 1), 1:57],
                                     p[:].rearrange("p (a b) -> p a b", b=56),
                                     AF.Identity, bias=bq_[oc][:], scale=1.0)
            for j in range(5):      # halo q: 5 chunks of 6 rows (336 cols)
                p = pst(f"qhp_{oc}_{j}", f"psQK{j % 2}", 336)
                for kc in range(2):
                    nc.tensor.matmul(
                        p[:], wqt[kc][:, 128 * oc:128 * (oc + 1)],
                        xh[kc][:, 336 * j:336 * (j + 1)],
                        start=(kc == 0), stop=False)
                # masked bias row: adds bq only on non-pad halo rows
                nc.tensor.matmul(p[:], bqrow[:, 128 * oc:128 * (oc + 1)].bitcast(F32),
                                 xb[:, 336 * j:336 * (j + 1)].bitcast(F32),
                                 start=False, stop=True)
                nc.scalar.activation(qhv[:, 6 * j:6 * (j + 1), 1:57],
                                     p[:].rearrange("p (a b) -> p a b", b=56),
                                     AF.Identity)

        # ------------- offset conv + LN + GELU + pw (per cc to share tiles) ------
        off_sb = [sb.tile([4, N], F32, name=f"off{cc}") for cc in range(2)]
        for cc in range(2):
            convs = sb.tile([128, N], F32R, name=f"convs{cc}", tag="convs", bufs=1)
            convsq = sb.tile([128, N], F32R, name=f"convsq{cc}", tag="convsq", bufs=1)
            gl = sb.tile([128, N], F32R, name=f"gl{cc}", tag="gl", bufs=1)
            qv = qf[cc][:, :].rearrange("p (i r j s) -> p i r j s", r=2, s=2, j=29)
            for half in range(2):
                p = pst(f"ocp_{cc}_{half}", f"psQK{half}")
                for t, (dy, dx) in enumerate(TAPS):
                    tap = dy * 3 + dx
                    ib = 0 if dy < 2 else 1
                    r = dy if dy < 2 else 0
                    jb = 0 if dx < 2 else 1
                    s_ = dx if dx < 2 else 0
                    i0 = half * 14 + ib
                    rhs = qv[:, i0:i0 + 14, r, jb:jb + 28, s_]
                    nc.tensor.matmul(p[:], codiag[:, (cc * 9 + tap) * 128:(cc * 9 + tap + 1) * 128],
                                     rhs, start=(t == 0), stop=(t == len(TAPS) - 1))
                nc.scalar.activation(convs[:, MC * half:MC * (half + 1)], p[:], AF.Identity)
                nc.scalar.square(convsq[:, MC * half:MC * (half + 1)], p[:])
            mean = sb.tile([2, N], F32R, name=f"mean{cc}", tag="mean", bufs=1)
            rstd = sb.tile([2, N], F32R, name=f"rstd{cc}", tag="rstd", bufs=1)
            for half in range(2):
                sl = slice(MC * half, MC * (half + 1))
                psum_s = pst(f"lns_{cc}_{half}", "psB", MC, 2)
                psum_q = pst(f"lnq_{cc}_{half}", "psC", MC, 2)
                nc.tensor.matmul(psum_s[:], ind2[:], convs[:, sl], start=True, stop=True)
                nc.tensor.matmul(psum_q[:], ind2[:], convsq[:, sl], start=True, stop=True)
                m_ = sb.tile([2, MC], F32, name=f"m_{cc}_{half}", tag="m_", bufs=1)
                v_ = sb.tile([2, MC], F32, name=f"v_{cc}_{half}", tag="v_", bufs=1)
                nc.vector.tensor_scalar(m_[:], psum_s[:], 1.0 / 64, None, op0=ALU.mult)
                nc.vector.tensor_scalar(v_[:], psum_q[:], 1.0 / 64, None, op0=ALU.mult)
                m2 = sb.tile([2, MC], F32, name=f"m2_{cc}_{half}", tag="m2", bufs=1)
                nc.vector.tensor_mul(m2[:], m_[:], m_[:])
                nc.vector.tensor_sub(v_[:], v_[:], m2[:])
                sd = sb.tile([2, MC], F32, name=f"sd_{cc}_{half}", tag="sd", bufs=1)
                nc.scalar.activation(sd[:], v_[:], AF.Sqrt, bias=epsc[:2, :], scale=1.0)
                nc.vector.tensor_copy(mean[:, sl], m_[:])
                nc.vector.reciprocal(rstd[:, sl], sd[:])
            for half in range(2):
                sl = slice(MC * half, MC * (half + 1))
                pbm = pst(f"bm_{cc}_{half}", "psB")
                pbr = pst(f"br_{cc}_{half}", "psC")
                nc.tensor.matmul(pbm[:], e2[:], mean[:, sl], start=True, stop=True)
                nc.tensor.matmul(pbr[:], e2[:], rstd[:, sl], start=True, stop=True)
                xm = sb.tile([128, MC], F32, name=f"xm_{cc}_{half}", tag="xm", bufs=1)
                nc.vector.tensor_sub(xm[:], convs[:, sl].bitcast(F32), pbm[:])
                nc.vector.tensor_mul(xm[:], xm[:], pbr[:])
                xn = sb.tile([128, MC], F32, name=f"xn_{cc}_{half}", tag="xn", bufs=1)
                nc.vector.tensor_scalar(xn[:], xm[:], coln[:, 0:1], coln[:, 1:2],
                                        op0=ALU.mult, op1=ALU.add)
                nc.scalar.activation(gl[:, sl], xn[:], AF.Gelu)
            for half in range(2):
                sl = slice(MC * half, MC * (half + 1))
                ppw = pst(f"pw_{cc}_{half}", "psB", MC, 4)
                nc.tensor.matmul(ppw[:], copw[:, 4 * cc:4 * cc + 4], gl[:, sl],
                                 start=True, stop=True)
                nc.vector.tensor_copy(off_sb[cc][:, sl], ppw[:])

        # ------------- ab path -------------
        pab = pst("pab", "psB", HK, 64)
        szv = sz[:, :].rearrange("p (l s) -> p l s", s=2)
        for t, d in enumerate((1, 0, 2)):
            ib = 0 if d < 2 else 1
            s_ = d if d < 2 else 0
            rhs = szv[:, ib:ib + HK, s_]
            nc.tensor.matmul(pab[:], abdiag[:, 64 * d:64 * (d + 1)].bitcast(F32),
                             rhs.bitcast(F32), start=(t == 0), stop=(t == 2))
        abc = sb.tile([64, HK], F32R, name="abc")
        abcq = sb.tile([64, HK], F32R, name="abcq")
        nc.scalar.activation(abc[:], pab[:], AF.Identity)
        nc.scalar.square(abcq[:], pab[:])
        pas = pst("pas", "psB", HK, 4)
        paq = pst("paq", "psC", HK, 4)
        nc.tensor.matmul(pas[:], ind416[:].bitcast(F32), abc[:].bitcast(F32), start=True, stop=True)
        nc.tensor.matmul(paq[:], ind416[:].bitcast(F32), abcq[:].bitcast(F32), start=True, stop=True)
        abm = sb.tile([4, HK], F32R, name="abm")
        abv = sb.tile([4, HK], F32, name="abv")
        nc.vector.tensor_scalar(abm[:], pas[:], 1.0 / SGC, None, op0=ALU.mult)
        nc.vector.tensor_scalar(abv[:], paq[:], 1.0 / SGC, None, op0=ALU.mult)
        abm2 = sb.tile([4, HK], F32, name="abm2")
        nc.vector.tensor_mul(abm2[:], abm[:].bitcast(F32), abm[:].bitcast(F32))
        nc.vector.tensor_sub(abv[:], abv[:], abm2[:])
        absd = sb.tile([4, HK], F32, name="absd")
        nc.scalar.activation(absd[:], abv[:], AF.Sqrt, bias=epsc[:4, :], scale=1.0)
        abrs = sb.tile([4, HK], F32R, name="abrs")
        nc.vector.reciprocal(abrs[:], absd[:])
        pabm = pst("pabm", "psB", HK, 64)
        pabr = pst("pabr", "psC", HK, 64)
        nc.tensor.matmul(pabm[:], e416[:].bitcast(F32), abm[:].bitcast(F32), start=True, stop=True)
        nc.tensor.matmul(pabr[:], e416[:].bitcast(F32), abrs[:].bitcast(F32), start=True, stop=True)
        abxm = sb.tile([64, HK], F32, name="abxm")
        nc.vector.tensor_sub(abxm[:], abc[:].bitcast(F32), pabm[:])
        nc.vector.tensor_mul(abxm[:], abxm[:], pabr[:])
        nc.vector.tensor_scalar(abxm[:], abxm[:], abln[:, 0:1], abln[:, 1:2],
                                op0=ALU.mult, op1=ALU.add)
        abgl = sb.tile([64, HK], F32R, name="abgl")
        nc.scalar.activation(abgl[:], abxm[:], AF.Gelu)
        pab0 = pst("pab0", "psB", HK, 4)
        pab1 = pst("pab1", "psC", HK, 4)
        nc.tensor.matmul(pab0[:], abpw[:, 0:4].bitcast(F32), abgl[:].bitcast(F32), start=True, stop=True)
        nc.tensor.matmul(pab1[:], abpw[:, 4:8].bitcast(F32), abgl[:].bitcast(F32), start=True, stop=True)

        # ------------- pos = clip(off + ab + ref) -------------
        pos = [sb.tile([4, N], F32, name=f"pos{cc}") for cc in range(2)]
        pabs = [pab0, pab1]
        for cc in range(2):
            t = sb.tile([4, N], F32, name=f"post{cc}")
            ab_b = pabs[cc][:, :].rearrange("p (a b) -> p a b", a=1).to_broadcast([4, HK, WK])
            nc.vector.tensor_tensor(t[:].rearrange("p (a b) -> p a b", b=WK),
                                    off_sb[cc][:, :].rearrange("p (a b) -> p a b", b=WK),
                                    ab_b, op=ALU.add)
            nc.vector.tensor_add(t[:], t[:], ref4[:])
            nc.vector.tensor_scalar(pos[cc][:], t[:], 1.0, -1.0, op0=ALU.min, op1=ALU.max)

        # ------------- transpose pos -> posT [112, 7*8] -------------
        posT = sb.tile([112, 56], F32, name="posT")
        for j in range(7):
            for cc in range(2):
                pt = pst(f"pt_{j}_{cc}", "psB", 4, 112)
                nc.tensor.transpose(pt[:], pos[cc][:, 112 * j:112 * (j + 1)], idn[:4, :4])
                nc.vector.tensor_copy(posT[:, j * 8 + 4 * cc:j * 8 + 4 * cc + 4], pt[:])

        # ------------- index & bilinear weights ([112, 28], slot s=j*4+g) ----------
        pv = posT[:, :].rearrange("p (j g o) -> p j g o", g=4, o=2)
        pgy = sb.tile([112, 28], F32, name="pgy")
        pgx = sb.tile([112, 28], F32, name="pgx")
        nc.vector.tensor_scalar(pgy[:].rearrange("p (a b) -> p a b", a=7), pv[:, :, :, 0],
                                1.0, 27.5, op0=ALU.add, op1=ALU.mult)
        nc.vector.tensor_scalar(pgx[:].rearrange("p (a b) -> p a b", a=7), pv[:, :, :, 1],
                                1.0, 27.5, op0=ALU.add, op1=ALU.mult)

        def floor_(x, nm):
            xi = sb.tile([112, 28], I32, name=nm + "i")
            nc.vector.tensor_copy(xi[:], x[:])
            xr = sb.tile([112, 28], F32, name=nm + "r")
            nc.vector.tensor_copy(xr[:], xi[:])
            gt = sb.tile([112, 28], F32, name=nm + "g")
            nc.vector.tensor_tensor(gt[:], xr[:], x[:], op=ALU.is_gt)
            fl = sb.tile([112, 28], F32, name=nm + "f")
            nc.vector.tensor_sub(fl[:], xr[:], gt[:])
            return fl

        y0 = floor_(pgy, "fy")
        x0 = floor_(pgx, "fx")
        wy = sb.tile([112, 28], F32, name="wy")
        wx = sb.tile([112, 28], F32, name="wx")
        nc.vector.tensor_sub(wy[:], pgy[:], y0[:])
        nc.vector.tensor_sub(wx[:], pgx[:], x0[:])
        idxf = sb.tile([112, 28], F32, name="idxf")
        nc.vector.tensor_scalar(idxf[:], y0[:], float(W), None, op0=ALU.mult)
        nc.vector.tensor_add(idxf[:], idxf[:], gbase[:])
        nc.vector.tensor_add(idxf[:], idxf[:], x0[:])
        idxi = sb.tile([112, 28], I32, name="idxi")
        nc.vector.tensor_copy(idxi[:], idxf[:])
        ay = sb.tile([112, 28], F32, name="ay")
        axx = sb.tile([112, 28], F32, name="axx")
        nc.vector.tensor_scalar(ay[:], wy[:], -1.0, 1.0, op0=ALU.mult, op1=ALU.add)
        nc.vector.tensor_scalar(axx[:], wx[:], -1.0, 1.0, op0=ALU.mult, op1=ALU.add)
        w00 = sb.tile([112, 28], F32, name="w00")
        w10 = sb.tile([112, 28], F32, name="w10")
        w01 = sb.tile([112, 28], F32, name="w01")
        w11 = sb.tile([112, 28], F32, name="w11")
        nc.vector.tensor_mul(w00[:], ay[:], axx[:])
        nc.vector.tensor_mul(w10[:], wy[:], axx[:])
        nc.vector.tensor_mul(w01[:], ay[:], wx[:])
        nc.vector.tensor_mul(w11[:], wy[:], wx[:])

        # ------------- gather + bilinear (two halves of 14 slots) -------------
        xs_pt = sb.tile([112, 28, 64], F32, name="xs_pt", tag="bigq", bufs=2)
        for sh in range(2):
            gth = sb.tile([112, 14, 256], F32, name=f"gth{sh}", tag="bigq", bufs=2)
            for s0 in range(14):
                s = sh * 14 + s0
                nc.gpsimd.indirect_dma_start(
                    out=gth[:, s0, :], out_offset=None, in_=XT2[:],
                    in_offset=bass.IndirectOffsetOnAxis(ap=idxi[:, s:s + 1], axis=0))
            ssl = slice(sh * 14, sh * 14 + 14)

            def corner(a, b_):
                return gth[:, :, 128 * b_ + 64 * a:128 * b_ + 64 * a + 64]

            def wbc(wt):
                return wt[:, ssl].rearrange("p (a b) -> p a b", b=1).to_broadcast([112, 14, 64])

            tmpb = sb.tile([112, 14, 64], F32, name=f"tmpb{sh}", tag="tmpb")
            xp = xs_pt[:, ssl, :]
            nc.vector.tensor_tensor(xp, corner(0, 0), wbc(w00), op=ALU.mult)
            nc.vector.tensor_tensor(tmpb[:], corner(1, 0), wbc(w10), op=ALU.mult)
            nc.vector.tensor_add(xp, xp, tmpb[:])
            nc.vector.tensor_tensor(tmpb[:], corner(0, 1), wbc(w01), op=ALU.mult)
            nc.vector.tensor_add(xp, xp, tmpb[:])
            nc.vector.tensor_tensor(tmpb[:], corner(1, 1), wbc(w11), op=ALU.mult)
            nc.vector.tensor_add(xp, xp, tmpb[:])

        # ------------- transpose xs_pt -> xs2 [65, 3136] (+ones row) -------------
        xs2 = sb.tile([65, G * N], F32R, name="xs2")
        nc.sync.dma_start(xs2[64:65, :], ONESR[:])
        for s in range(28):
            j, g = s // 4, s % 4
            pxs = pst(f"pxs{s}", "psB" if s % 2 == 0 else "psC", 112, 64)
            nc.tensor.transpose(pxs[:], xs_pt[:, s, :], idn[:112, :112])
            nc.vector.tensor_copy(xs2[0:64, g * N + j * 112:g * N + j * 112 + 112], pxs[:])

        # ------------- k projection -------------
        kt = [sb.tile([128, N], F32R, name=f"kt{oc}") for oc in range(2)]
        for oc in range(2):
            for nh in range(2):
                p = pst(f"kp_{oc}_{nh}", f"psQK{nh}")
                for g in range(G):
                    nc.tensor.matmul(p[:], wktb[g][:, 128 * oc:128 * (oc + 1)],
                                     xs2[:, g * N + MC * nh:g * N + MC * (nh + 1)],
                                     start=(g == 0), stop=(g == G - 1))
                nc.scalar.activation(kt[oc][:, MC * nh:MC * (nh + 1)], p[:], AF.Identity)

        # ------------- vT projection (bf16) -------------
        vt = []
        for j in range(7):
            p = pst(f"vp{j}", f"psQK{2 + (j % 2)}", C)
            for g in range(G):
                nc.tensor.matmul(p[:NSZ[j], :], xs2[:, g * N + 128 * j:g * N + 128 * j + NSZ[j]],
                                 wvtb[g][:], start=(g == 0), stop=(g == G - 1))
            v = sb.tile([128, C], FP16, name=f"vt{j}")
            nc.scalar.activation(v[:NSZ[j], :], p[:NSZ[j], :], AF.Identity)
            vt.append(v)

        # ------------- attention + lepe + out-proj -------------
        yt = [sb.tile([128, MH], F32, name=f"yt{oc}", tag="bigx", bufs=2) for oc in range(2)]
        for mc in range(NMC):
            att_out = []
            for hg in range(2):
                pav = pst(f"pav_{mc}_{hg}", "psAV")
                psm = pst(f"psm_{mc}_{hg}", "psSM")
                for j in range(7):
                    nj = NSZ[j]
                    pqk = [pst(f"pqk_{mc}_{hg}_{j}_{h}", f"psQK{h}") for h in range(4)]
                    for h in range(4):
                        nc.tensor.matmul(
                            pqk[h][:nj, :],
                            kt[hg][32 * h:32 * (h + 1), 128 * j:128 * j + nj],
                            qh[hg][:, :].rearrange("p (a b) -> p a b", b=58)[
                                32 * h:32 * (h + 1), 1 + mc * 7:1 + mc * 7 + 7, 1:57],
                            start=True, stop=True, tile_position=(32 * h, 0))
                    ex = [sb2.tile([128, MC], FP16, name=f"ex_{mc}_{hg}_{j}_{h}", tag=f"ex{h}")
                          for h in range(4)]
                    for h in range(4):
                        nc.scalar.activation(ex[h][:nj, :], pqk[h][:nj, :], AF.Exp, scale=SCALE)
                    for h in range(4):
                        nc.tensor.matmul(
                            pav[32 * h:32 * (h + 1), :],
                            vt[j][:nj, 128 * hg + 32 * h:128 * hg + 32 * (h + 1)],
                            ex[h][:nj, :],
                            start=(j == 0), stop=(j == 6), tile_position=(0, 32 * h),
                            skip_group_check=True)
                    for h in range(4):
                        nc.tensor.matmul(
                            psm[32 * h:32 * (h + 1), :],
                            onesum[:nj, :],
                            ex[h][:nj, :],
                            start=(j == 0), stop=(j == 6), tile_position=(0, 32 * h),
                            skip_group_check=True)
                # lepe depthwise 3x3 (stride 1) on halo q
                plp = pst(f"plp_{mc}_{hg}", "psQK0")
                qhv2 = qh[hg][:, :].rearrange("p (r w) -> p r w", w=58)
                for t, (dy, dx) in enumerate(TAPS):
                    tap = dy * 3 + dx
                    rhs = qhv2[:, mc * 7 + dy:mc * 7 + dy + 7, dx:dx + 56]
                    nc.tensor.matmul(plp[:],
                                     rpediag[:, (hg * 9 + tap) * 128:(hg * 9 + tap + 1) * 128],
                                     rhs, start=(t == 0), stop=(t == len(TAPS) - 1))
                rec = sb.tile([128, MC], F32, name=f"rec_{mc}_{hg}", tag="rec")
                nc.vector.reciprocal(rec[:], psm[:])
                t1 = sb.tile([128, MC], F32, name=f"t1_{mc}_{hg}", tag="t1")
                nc.vector.tensor_mul(t1[:], pav[:], rec[:])
                nc.vector.tensor_add(t1[:], t1[:], plp[:])
                ao = sb.tile([128, MC], F32R, name=f"ao_{mc}_{hg}", tag=f"ao{hg}")
                nc.vector.tensor_scalar(ao[:], t1[:], rpeb_[hg][:], None, op0=ALU.add)
                att_out.append(ao)
            for oc in range(2):
                p = pst(f"yp_{mc}_{oc}", "psQK1")
                for kc in range(2):
                    nc.tensor.matmul(p[:], wot[kc][:, 128 * oc:128 * (oc + 1)],
                                     att_out[kc][:], start=(kc == 0), stop=(kc == 1))
                nc.vector.tensor_scalar(yt[oc][:, mc * MC:(mc + 1) * MC], p[:], bo_[oc][:],
                                        None, op0=ALU.add)

        nc.gpsimd.dma_start(Y[0:128, :], yt[0][:])
        nc.gpsimd.dma_start(Y[128:256, :], yt[1][:])

    nc.compile()
    return nc


# ---------------- host side ----------------

def prep_inputs(x, Str_Zab, co_dw_w, co_dw_b, co_ln_g, co_ln_b, co_pw_w,
                ab_dw_w, ab_dw_b, ab_ln_g, ab_ln_b, ab_pw_w,
                wq, bq, wk, bk, wv, bv, wo, bo, rpe_w, rpe_b):
    import ml_dtypes
    f = np.float32
    x = np.asarray(x, f)
    Str_Zab = np.asarray(Str_Zab, f)

    WQT = np.ascontiguousarray(np.asarray(wq, f).T)
    WOT = np.ascontiguousarray(np.asarray(wo, f).T)
    WKTB = np.zeros((G, 65, C), f)
    WVTB = np.zeros((G, 65, C), f)
    wkT = np.asarray(wk, f).T
    wvT = np.asarray(wv, f).T
    for g in range(G):
        WKTB[g, :64] = wkT[64 * g:64 * (g + 1)]
        WVTB[g, :64] = wvT[64 * g:64 * (g + 1)]
    WKTB[G - 1, 64] = np.asarray(bk, f)
    WVTB[G - 1, 64] = np.asarray(bv, f)
    BQ = np.asarray(bq, f).reshape(C, 1)
    BO = np.asarray(bo, f).reshape(C, 1)
    RPEB = np.asarray(rpe_b, f).reshape(C, 1)

    co_dw = np.asarray(co_dw_w, f).reshape(NGC, 9)
    RPED = np.asarray(rpe_w, f).reshape(C, 9)
    CODIAG = np.zeros((128, 18 * 128), f)
    RPEDIAG = np.zeros((128, 18 * 128), f)
    ch = np.arange(128)
    for cc in range(2):
        for tap in range(9):
            blk = slice((cc * 9 + tap) * 128, (cc * 9 + tap + 1) * 128)
            d = np.zeros((128, 128), f)
            d[ch, ch] = co_dw[(ch % 64), tap]
            CODIAG[:, blk] = d
            d2 = np.zeros((128, 128), f)
            d2[ch, ch] = RPED[cc * 128 + ch, tap]
            RPEDIAG[:, blk] = d2

    COLN = np.zeros((128, 2), f)
    COLN[:, 0] = np.tile(np.asarray(co_ln_g, f), 2)
    COLN[:, 1] = np.tile(np.asarray(co_ln_b, f), 2)
    COPW = np.zeros((128, 8), f)
    pw = np.asarray(co_pw_w, f)
    for cc in range(2):
        for gl in range(2):
            for o in range(2):
                COPW[64 * gl:64 * (gl + 1), 4 * cc + 2 * gl + o] = pw[o]
    ABDIAG = np.zeros((64, 3 * 64), f)
    abw = np.asarray(ab_dw_w, f).reshape(SGC, 3)
    c64 = np.arange(64)
    for d in range(3):
        m = np.zeros((64, 64), f)
        m[c64, c64] = abw[c64 % SGC, d]
        ABDIAG[:, 64 * d:64 * (d + 1)] = m
    ABLN = np.zeros((64, 2), f)
    ABLN[:, 0] = np.tile(np.asarray(ab_ln_g, f), 4)
    ABLN[:, 1] = np.tile(np.asarray(ab_ln_b, f), 4)
    ABPW = np.zeros((64, 8), f)
    apw = np.asarray(ab_pw_w, f)
    for g in range(G):
        for o in range(2):
            ABPW[16 * g:16 * (g + 1), 2 * g + o] = apw[o]
    IND2 = np.zeros((128, 2), f)
    IND2[:64, 0] = 1; IND2[64:, 1] = 1
    E2 = np.zeros((2, 128), f)
    E2[0, :64] = 1; E2[1, 64:] = 1
    IND416 = np.zeros((64, 4), f)
    E416 = np.zeros((4, 64), f)
    for g in range(G):
        IND416[16 * g:16 * (g + 1), g] = 1
        E416[g, 16 * g:16 * (g + 1)] = 1
    ONESUM = np.ones((128, 32), np.float16)
    ry = ((np.linspace(0.5, HK - 0.5, HK) / (HK - 1.0)) * 2.0 - 1.0).astype(f)
    rx = ((np.linspace(0.5, WK - 0.5, WK) / (WK - 1.0)) * 2.0 - 1.0).astype(f)
    REF4 = np.zeros((4, N), f)
    REF4[0] = REF4[2] = np.repeat(ry, WK)
    REF4[1] = REF4[3] = np.tile(rx, HK)
    GBASE = np.zeros((112, 28), f)
    for s in range(28):
        GBASE[:, s] = (s % 4) * HW
    IDN = np.eye(128, dtype=f)

    shared = dict(WQT=WQT, WKTB=WKTB, WVTB=WVTB, WOT=WOT, BQ=BQ, BO=BO, RPEB=RPEB,
                  CODIAG=CODIAG, RPEDIAG=RPEDIAG, COLN=COLN, COPW=COPW,
                  ABDIAG=ABDIAG, ABLN=ABLN, ABPW=ABPW, IND2=IND2, E2=E2,
                  IND416=IND416, E416=E416, ONESUM=ONESUM, REF4=REF4,
                  GBASE=GBASE, IDN=IDN, EPSC=np.full((128, 1), EPS, f),
                  BQROW=np.asarray(bq, f).reshape(1, C),
                  ZROW=np.zeros((128, 64), f), ONESR=np.ones((1, G * N), f))

    in_maps = []
    for core in range(8):
        b, hf = core // 2, core % 2
        XFb = np.ascontiguousarray(x[b].reshape(C, HW))
        img = x[b].reshape(C, H, W)
        XHw = np.zeros((C, 30, W), f)
        if hf == 0:
            XHw[:, 1:30] = img[:, 0:29]
        else:
            XHw[:, 0:29] = img[:, 27:56]
        SZp = np.zeros((STR_DIM, 58), f)
        SZp[:, 1:57] = Str_Zab[b]
        xg = x[b].reshape(G, NGC, H, W)
        XT2v = np.zeros((TROWS, 128), f)
        t = XT2v[:G * HW].reshape(G, H, W, 128)
        t[:, :, :, :64] = np.moveaxis(xg, 1, -1)
        t[:, :H - 1, :, 64:] = np.moveaxis(xg[:, :, 1:, :], 1, -1)
        XBv = np.zeros((1, 30 * W), f)
        xbv = XBv.reshape(30, W)
        if hf == 0:
            xbv[1:30] = 1.0
        else:
            xbv[0:29] = 1.0
        in_maps.append(dict(XF=XFb, XH=XHw.reshape(C, 30 * W), XT2=XT2v, XB=XBv,
                            SZ=SZp, **shared))
    return in_maps


_NC_CACHE = {}


def get_program():
    if "nc" not in _NC_CACHE:
        _NC_CACHE["nc"] = build_program()
    return _NC_CACHE["nc"]


def kernel(**inputs):
    from concourse.bass_utils import run_bass_kernel_spmd
    nc = get_program()
    in_maps = prep_inputs(**{k: np.asarray(v) for k, v in inputs.items()})
    res = run_bass_kernel_spmd(nc, in_maps, core_ids=list(range(8)))
    out = np.zeros((B, C, HW), np.float32)
    for core in range(8):
        b, hf = core // 2, core % 2
        out[b, :, hf * MH:(hf + 1) * MH] = res.results[core]["Y"]
    return out.reshape(B, C, H, W)
